# revision 40
# baseline (speedup 1.0000x reference)
"""Trainium2 Bass kernel for an EoMT transformer encoder layer.

Layer (per batch element):
    xn  = LN1(x);  qkv = xn @ qkv_w;  masked softmax attention (16 heads);
    y   = attn_out @ proj_w + proj_b;  x1 = x + y
    h   = gelu(LN2(x1) @ fc1_w + fc1_b);  y2 = h @ fc2_w + fc2_b; out = x1 + y2

Sharding: pure data-parallel over batch — B=8 maps 1:1 onto the 8 NeuronCores,
no collectives.  Each core runs the full layer for its batch element.

Per-core design (measured on hw; see git of this file for the journey):
  - q,k in TRANSPOSED layout qk^T [2D, S]; v in NATURAL layout [S, D] with a
    ones column appended per head so the softmax denominator falls out of the
    attn@v matmul (psum row 64).
  - scoresT [k, q] = k^T.T @ q^T per (head, k-tile), flash-style:
    scores -> exp (ACT reads PSUM, writes bf16 SBUF) -> mask-mul -> attn@v.
    No max-subtraction (|score*scale| < ~3 by construction).
  - The TRN2 PE drops to mid p-state (1.2 GHz) whenever it idles, so the
    attention phase is emitted as a gapless pipeline: the 1024 patch
    q-columns of each k-tile accumulate in a 2-bank psum ring and get ONE
    exp; the 100 query q-columns of THREE k-tiles share a 1-bank psum and
    ONE exp (amortizing the ~260ns/instr ACT overhead); remaining qk^T
    tiles are interleaved as PE filler through the sc ring (never the "tr"
    slot, which the 3-k-tile query psum owns — sharing it deadlocks the
    in-order PE).
  - qk, v and fc1 run in fp8e4 DoubleRow (ko-pairs, contract 256):
    weights host-prescaled x64 into e4m3 range; activations are cast to
    e4m3 unscaled (LN outputs are ~N(0,1), in-range).  The 1/64^2 lands in
    the exp scale (qk), the ones-column=64 denominator (v), and the gelu
    activation scale (fc1).  fc2 stays bf16: fc1+fc2 both fp8 measures
    2.4e-2 frobenius (> the 2e-2 gate; each alone ~1.7e-2, noise adds in
    quadrature — hw matches a host numpy model to 5e-5).  DoubleRow
    stationary APs need pair-step%16==0, hence the S->1136 padded fp8
    transposed tiles.
  - LN statistics on DVE (bn_stats), normalize on ACT (Identity with
    per-partition scale/bias APs); x tiles DMA ahead of the weight flood.
  - 1/den computed as exp(-ln(den)) on ACT (DVE reciprocal is ~7.7ns/elem),
    then partition-broadcast on the PE via a constant selector matmul
    (a 64-descriptor stride-0 DMA broadcast costs ~2us/strip and paced
    the whole proj phase), one multiply per outT strip.
  - fc1 emits h^T [MLP, S] with gelu+bias fused into the PSUM->SBUF
    activation; fc2 contracts h^T back to natural [S, D]; proj/fc1/fc2
    accumulate through the shared 2-deep psum ring in 512-col chunks.
  - LN2's fp8 transposes are deferred two s-tiles so the PE never waits
    on the serial proj-drain -> LN2 chain of the same tile.
Matmuls bf16 except qk/v/fc1 (fp8 DoubleRow); fp32 PSUM.
"""

import os
import sys

for _p in ("/opt/trn_rl_repo", "/root/.axon_site/_ro/trn_rl_repo"):
    if _p not in sys.path and os.path.isdir(_p):
        sys.path.append(_p)

import numpy as np
import ml_dtypes

import concourse.bass as bass
import concourse.tile as tile
from concourse import bacc
from concourse import mybir
from concourse.masks import make_identity

AFT = mybir.ActivationFunctionType
ALU = mybir.AluOpType
BF16 = mybir.dt.bfloat16
F32 = mybir.dt.float32
F8 = mybir.dt.float8e4

P = 128


class Cfg:
    def __init__(self, B=8, S=1124, D=1024, NP=1024, NQ=100, MLP=4096,
                 EPS=1e-6, use_ln1_g=False, use_ln1_b=False, use_ln2_g=False,
                 use_ln2_b=False, use_proj_b=False, use_fc2_b=False,
                 gelu=True):
        self.B, self.S, self.D = B, S, D
        self.NP, self.NQ, self.MLP, self.EPS = NP, NQ, MLP, EPS
        self.DH = 64
        self.H = D // self.DH
        assert D % P == 0 and MLP % P == 0
        self.SCALE = self.DH ** -0.5
        self.use_ln1_g, self.use_ln1_b = use_ln1_g, use_ln1_b
        self.use_ln2_g, self.use_ln2_b = use_ln2_g, use_ln2_b
        self.use_proj_b, self.use_fc2_b = use_proj_b, use_fc2_b
        self.gelu = gelu

    def key(self):
        return tuple(sorted((k, v) for k, v in self.__dict__.items()))


def _s_tiles(S):
    return [(i * P, min(P, S - i * P)) for i in range((S + P - 1) // P)]


def _chunks(N, width=512):
    return [(i * width, min(width, N - i * width))
            for i in range((N + width - 1) // width)]


def build_layer(nc, cfg, io):
    """Trace the layer program into `nc`.  `io` maps names to DRAM APs."""
    S, D, H, MLP, NP, NQ = cfg.S, cfg.D, cfg.H, cfg.MLP, cfg.NP, cfg.NQ
    ND = D // P                      # contraction chunks of D
    NQK = 2 * D // P                 # m-tiles of transposed q|k
    NM = MLP // P                    # m-tiles of MLP hidden
    stiles = _s_tiles(S)
    NS = len(stiles)
    qch = _chunks(S)                 # free chunks of S, <=512, bank-aligned
    dch = _chunks(D)                 # free chunks of D

    WTW = 1024 if (3 * D) % 1024 == 0 else 3 * D   # qkv weight tile width
    n_qkvw = ND * (3 * D // WTW)
    WBUFS = max(n_qkvw + ND, NM) + 2   # qkv tiles + proj tiles live together

    x_d, out_d, maskT_d = io["x"], io["out"], io["maskT"]

    with tile.TileContext(nc) as tc:
        with (
            tc.tile_pool(name="const", bufs=1) as cpool,
            tc.tile_pool(name="rp", bufs=1) as rp,
            tc.tile_pool(name="wp", bufs=1) as wp,
            tc.tile_pool(name="st", bufs=1) as st,
            tc.tile_pool(name="dp", bufs=1, space="DRAM") as dp,
            tc.tile_pool(name="ps", bufs=1, space="PSUM") as ps,
        ):
            RT = dict(tag="r", bufs=39)
            WT = dict(tag="w", bufs=WBUFS)

            x1_d = [dp.tile([rows, D], F32, name=f"x1_scr{i}", tag=f"x1{i}")
                    for i, (s0, rows) in enumerate(stiles)]
            den_d = [dp.tile([1, S], BF16, name=f"den_scr{h}", tag=f"den{h}")
                     for h in range(H)]

            ident = cpool.tile([P, P], BF16, name="ident")
            make_identity(nc, ident[:])
            ident8 = cpool.tile([P, P], F8, name="ident8")
            nc.vector.tensor_copy(out=ident8[:], in_=ident[:])
            eps_t = cpool.tile([P, 1], F32, name="eps")
            nc.vector.memset(eps_t, cfg.EPS)
            negone_t = cpool.tile([P, 1], F32, name="negone")
            nc.vector.memset(negone_t, -1.0)


            # ---- x tiles first on the gpsimd queue so LN1 is not starved
            # by the weight-DMA flood sharing the DMA engine pool
            x_pre = {}
            for kt in range(min(6, NS)):
                s0, srows = stiles[kt]
                xt = st.tile([P, D], F32, name=f"x_pre{kt}", tag="xf", bufs=6)
                # split each prefetched tile across two queues: halves the
                # DMA latency ahead of the first LN1 (the critical path)
                hrows = srows // 2
                nc.gpsimd.dma_start(out=xt[:hrows], in_=x_d[s0:s0 + hrows])
                nc.sync.dma_start(out=xt[hrows:srows],
                                    in_=x_d[s0 + hrows:s0 + srows])
                x_pre[kt] = xt

            # ---- weight DMAs, ordered so attention prerequisites land
            # first: v columns, then the fp8 qk pair-tiles (x64-prescaled on
            # the host; the 1/64^2 is folded into the exp scale), then proj.
            vw8 = []
            v_w4 = io["v_w"].rearrange("(kp t ki) n -> ki kp t n", ki=P, t=2)
            for kp in range(ND // 2):
                t = wp.tile([P, 2, D], mybir.dt.uint8, name=f"vw8_{kp}", **WT)
                nc.sync.dma_start(out=t[:], in_=v_w4[:, kp, :, :])
                vw8.append(t)
            NKP = ND // 2
            qkw8 = {}
            qk_w4 = io["qk_w"].rearrange("(kp t ki) n -> ki kp t n",
                                         ki=P, t=2)
            for half in range(2):
                for kp in range(NKP):
                    t = wp.tile([P, 2, D], mybir.dt.uint8,
                                name=f"qkw8_{kp}_{half}", **WT)
                    nc.sync.dma_start(
                        out=t[:],
                        in_=qk_w4[:, kp, :, half * D:(half + 1) * D])
                    qkw8[(kp, half)] = t

            projw = []
            proj_w3 = io["proj_w"].rearrange("(ko ki) n -> ki ko n", ki=P)
            for ko in range(ND):
                t = wp.tile([P, D], BF16, name=f"projw{ko}", **WT)
                nc.sync.dma_start(out=t[:], in_=proj_w3[:, ko, :])
                projw.append(t)

            def bcast_vec(name, ap_1d):
                t = cpool.tile([P, ap_1d.shape[0]], F32, name=name)
                src = bass.AP(tensor=ap_1d.tensor, offset=ap_1d.offset,
                              ap=[[0, P]] + list(ap_1d.ap))
                nc.sync.dma_start(out=t[:], in_=src)
                return t

            ln1_g = bcast_vec("ln1_g", io["ln1_g"]) if cfg.use_ln1_g else None
            ln1_b = bcast_vec("ln1_b", io["ln1_b"]) if cfg.use_ln1_b else None
            ln2_g = bcast_vec("ln2_g", io["ln2_g"]) if cfg.use_ln2_g else None
            ln2_b = bcast_vec("ln2_b", io["ln2_b"]) if cfg.use_ln2_b else None
            proj_b = bcast_vec("proj_b", io["proj_b"]) if cfg.use_proj_b else None
            fc2_b = bcast_vec("fc2_b", io["fc2_b"]) if cfg.use_fc2_b else None

            fc1_b_sb = cpool.tile([P, NM], F32, name="fc1_b_sb")
            nc.sync.dma_start(out=fc1_b_sb[:],
                              in_=io["fc1_b"].rearrange("(mo ki) -> ki mo", ki=P))


            # ---------------- LN + transpose helpers ----------------
            # stats on DVE, normalize on ACT (Identity w/ per-partition
            # scale=rstd, bias=-mean*rstd) so the two engines pipeline.
            def layer_norm(x_t, srows, g, b, name, out_dtype=BF16,
                           out_pool=None, out_kw=None):
                nsub = 2 if D > 512 else 1
                half = D // nsub
                stats = st.tile([P, nsub, 6], F32, name=f"sta{name}",
                                tag="stats", bufs=2)
                mv = st.tile([P, 2], F32, name=f"mv{name}", tag="mv", bufs=2)
                for i in range(nsub):
                    nc.vector.bn_stats(out=stats[:srows, i],
                                       in_=x_t[:srows, i * half:(i + 1) * half])
                nc.vector.bn_aggr(out=mv[:srows], in_=stats[:srows])
                std = st.tile([P, 1], F32, name=f"std{name}", tag="std", bufs=2)
                nc.scalar.activation(out=std[:srows], in_=mv[:srows, 1:2],
                                     func=AFT.Sqrt, bias=eps_t[:srows],
                                     scale=1.0)
                nc.vector.reciprocal(out=std[:srows], in_=std[:srows])
                neg = st.tile([P, 1], F32, name=f"neg{name}", tag="neg", bufs=2)
                nc.vector.tensor_scalar(out=neg[:srows], in0=mv[:srows, 0:1],
                                        scalar1=std[:srows, 0:1],
                                        scalar2=negone_t[:srows],
                                        op0=ALU.mult, op1=ALU.mult)
                kw = out_kw if out_kw is not None else dict(tag="xn", bufs=3)
                xn_t = (out_pool or st).tile([P, D], out_dtype,
                                             name=f"xn{name}", **kw)
                with nc.allow_low_precision(reason="fp8/bf16 LN output"):
                    nc.scalar.activation(out=xn_t[:srows], in_=x_t[:srows],
                                         func=AFT.Identity,
                                         bias=neg[:srows, 0:1],
                                         scale=std[:srows, 0:1])
                if g is not None:
                    nc.vector.tensor_mul(out=xn_t[:srows], in0=xn_t[:srows],
                                         in1=g[:srows])
                if b is not None:
                    nc.vector.tensor_add(out=xn_t[:srows], in0=xn_t[:srows],
                                         in1=b[:srows])
                return xn_t

            def transpose_into(xn_t, srows, s0, write, dtype=BF16):
                # alternate the transpose psum between the 1-bank "tr" slot
                # and the wider "sc" ring so back-to-back transposes pipeline
                for j in range(ND):
                    if j % 2 == 0:
                        pt = ps.tile([P, 512], dtype, name=f"ptr{j}", tag="tr",
                                     bufs=1)
                    else:
                        pt = ps.tile([P, 512], dtype, name=f"psr{j}", tag="sc",
                                     bufs=2)
                    idn = ident8 if dtype == F8 else ident
                    nc.tensor.transpose(pt[:P, :srows],
                                        xn_t[:srows, j * P:(j + 1) * P],
                                        idn[:srows, :srows])
                    write(j, pt[:P, :srows])

            def small_mm(lhsT_fn, rhs_fn, out_rows, chunks, copy_fn):
                """Accumulate over ko into a 1-bank psum per free chunk."""
                for (c0, cn) in chunks:
                    pm = ps.tile([P, 512], F32, name="ptr_mm", tag="tr", bufs=1)
                    for ko in range(ND):
                        nc.tensor.matmul(pm[:out_rows, :cn],
                                         lhsT=lhsT_fn(ko),
                                         rhs=rhs_fn(ko, c0, cn),
                                         start=(ko == 0), stop=(ko == ND - 1))
                    copy_fn(pm, c0, cn)

            # ---------------- LN1 + v, pipelined per s-tile ----------------
            # v natural [S, D] with interleaved ones column (attn@v stationary)
            SP8 = ((S + 15) // 16) * 16   # DoubleRow lhsT pair step %16
            xn8T = [rp.tile([P, 2, SP8], F8, name=f"xn8T{j}", **RT)
                    for j in range(ND // 2)]
            v_sb = []
            for kt in range(NS):
                t = rp.tile([P, H, 65], BF16, name=f"v{kt}", **RT)
                nc.vector.memset(t[:, :, 64:65], 64.0)
                v_sb.append(t)

            for kt, (s0, srows) in enumerate(stiles):
                if kt in x_pre:
                    x_t = x_pre[kt]
                else:
                    x_t = st.tile([P, D], F32, name="x_t", tag="xf", bufs=6)
                    nc.gpsimd.dma_start(out=x_t[:srows],
                                        in_=x_d[s0:s0 + srows])
                xn_t = layer_norm(x_t, srows, ln1_g, ln1_b, f"1_{s0}")

                def wr1(j, src_ap, s0=s0, srows=srows):
                    # bf16 transpose psum -> fp8 xn^T pairs, cast in the
                    # copy; no bf16 xn^T exists any more (v went fp8)
                    nc.vector.tensor_copy(
                        out=xn8T[j // 2][:, j % 2, s0:s0 + srows],
                        in_=src_ap)
                transpose_into(xn_t, srows, s0, wr1)

                def vcopy(pm, c0, cn, kt=kt, srows=srows):
                    h0 = c0 // 64
                    nc.vector.tensor_copy(
                        out=v_sb[kt][:srows, h0:h0 + cn // 64, 0:64],
                        in_=pm[:srows, :cn].rearrange("p (h d) -> p h d", d=64))
                for (c0, cn) in dch:
                    pm = ps.tile([P, 512], F32, name="ptr_mm", tag="tr",
                                 bufs=1)
                    for kp in range(ND // 2):
                        nc.tensor.matmul(
                            pm[:srows, :cn],
                            lhsT=xn8T[kp][:, :, s0:s0 + srows],
                            rhs=vw8[kp][:, :, c0:c0 + cn].bitcast(F8),
                            start=(kp == 0), stop=(kp == ND // 2 - 1),
                            perf_mode=mybir.MatmulPerfMode.DoubleRow)
                    vcopy(pm, c0, cn)

            # binarized transposed mask per (partially) masked k-tile
            # (emitted after LN1 so the mask DMAs + binarize don't block the
            # LN chain on the gpsimd/DVE queues at startup)
            mtiles = []
            for kt, (k0, krows) in enumerate(stiles):
                if k0 >= NP or NQ == 0:
                    mtiles.append(None)
                    continue
                mrows = min(k0 + krows, NP) - k0
                mf = st.tile([P, NQ], F32, name=f"mf{kt}", tag="mf", bufs=1)
                nc.gpsimd.dma_start(out=mf[:mrows], in_=maskT_d[k0:k0 + mrows])
                mb = st.tile([P, NQ], BF16, name=f"mb{kt}", tag="mb", bufs=NS - 1)
                nc.vector.tensor_scalar(out=mb[:mrows], in0=mf[:mrows],
                                        scalar1=0.5, scalar2=None,
                                        op0=ALU.is_gt)
                mtiles.append(mb)

            # ---------------- qk tiles: 0/ND upfront, rest as filler -------
            qkT = [None] * NQK

            def qk_unit(mt, c0, cn):
                """One chunk of one transposed qk tile (a PE filler unit)."""
                half, m0 = mt // ND, (mt % ND) * P
                def run():
                    t = qkT[mt]
                    # sc ring, NOT "tr": the pq triple psum owns "tr" across
                    # three k-tiles, so a filler there would deadlock the PE
                    pm = ps.tile([P, 512], F32, name="pqk_mm", tag="sc",
                                 bufs=2)
                    for kp in range(NKP):
                        nc.tensor.matmul(
                            pm[:P, :cn],
                            lhsT=qkw8[(kp, half)][:, :, m0:m0 + P].bitcast(F8),
                            rhs=xn8T[kp][:, :, c0:c0 + cn],
                            start=(kp == 0), stop=(kp == NKP - 1),
                            perf_mode=mybir.MatmulPerfMode.DoubleRow)
                    nc.vector.tensor_copy(out=t[:, c0:c0 + cn], in_=pm[:P, :cn])
                return run

            def alloc_qk(mt):
                qkT[mt] = rp.tile([P, S], BF16, name=f"qkT{mt}", **RT)

            HP = H // 2
            fillers = []
            for p in range(HP):
                for mt in (p, HP + p):
                    alloc_qk(mt)
            for mt in (0, HP):
                for (c0, cn) in qch:
                    qk_unit(mt, c0, cn)()
            for p in range(1, HP):
                for (c0, cn) in qch:
                    fillers.append(qk_unit(p, c0, cn))
                    fillers.append(qk_unit(HP + p, c0, cn))

            # ---------------- attention: gapless pipeline per head --------
            outT = [rp.tile([P, S], BF16, name=f"outT{j}", **RT)
                    for j in range(ND)]
            den_sb = st.tile([P, S], BF16, name="den_sb", tag="den", bufs=1)

            def emit_head(h):
                qbase, kbase = h * 64, D + h * 64
                qT = qkT[qbase // P][qbase % P:qbase % P + 64, :]
                kT = qkT[kbase // P][kbase % P:kbase % P + 64, :]
                po = ps.tile([P, 1536], F32, name=f"po{h}", tag="po", bufs=1)
                e_t = [None] * NS
                e2_t = [None] * NS   # (tile, col offset) for the query cols
                tri = {}

                def scores(kt):
                    # q-cols 0:1024 share a 2-bank psum and ONE exp (amortizes
                    # the ~260ns/instr ACT overhead); the 100 query cols of
                    # THREE k-tiles share the 1-bank "tr" slot and ONE exp.
                    k0, krows = stiles[kt]
                    e = rp.tile([P, NP], BF16, name=f"e{h}_{kt}", tag="expT",
                                bufs=3)
                    pm = ps.tile([P, 1024], F32, name="psc", tag="sc", bufs=2)
                    for (q0, qn) in qch[:2]:
                        nc.tensor.matmul(pm[:krows, q0:q0 + qn],
                                         lhsT=kT[:, k0:k0 + krows],
                                         rhs=qT[:, q0:q0 + qn],
                                         start=True, stop=True)
                    nc.scalar.activation(out=e[:krows, 0:NP],
                                         in_=pm[:krows, 0:NP],
                                         func=AFT.Exp, scale=cfg.SCALE / 4096.0)
                    e_t[kt] = e
                    q0, qn = qch[2]
                    tloc = kt % 3
                    if tloc == 0:
                        tri["pq"] = ps.tile([P, 512], F32, name="psq",
                                            tag="tr", bufs=1)
                        tri["e2"] = st.tile([P, 384], BF16, name=f"e2_{h}",
                                            tag="e2", bufs=2)
                        tri["kts"] = []
                    nc.tensor.matmul(tri["pq"][:krows,
                                               tloc * P:tloc * P + qn],
                                     lhsT=kT[:, k0:k0 + krows],
                                     rhs=qT[:, q0:q0 + qn],
                                     start=True, stop=True)
                    tri["kts"].append(kt)
                    e2_t[kt] = (tri["e2"], tloc * P)
                    if tloc == 2 or kt == NS - 1:
                        w = tloc * P + qn
                        nc.scalar.activation(out=tri["e2"][:P, 0:w],
                                             in_=tri["pq"][:P, 0:w],
                                             func=AFT.Exp,
                                             scale=cfg.SCALE / 4096.0)
                        for kt2 in tri["kts"]:
                            if mtiles[kt2] is None:
                                continue
                            k02, krows2 = stiles[kt2]
                            mrows = min(k02 + krows2, NP) - k02
                            et2, off = e2_t[kt2]
                            nc.vector.tensor_mul(
                                out=et2[:mrows, off:off + NQ],
                                in0=et2[:mrows, off:off + NQ],
                                in1=mtiles[kt2][:mrows])
                        # av for the query cols of the whole triple
                        for kt2 in tri["kts"]:
                            k02, krows2 = stiles[kt2]
                            et2, off = e2_t[kt2]
                            q0, qn = qch[2]
                            nc.tensor.matmul(
                                po[:65, 2 * 512:2 * 512 + qn],
                                lhsT=v_sb[kt2][:krows2, h, :],
                                rhs=et2[:krows2, off:off + qn],
                                start=(kt2 == 0), stop=(kt2 == NS - 1))

                def av(kt):
                    k0, krows = stiles[kt]
                    for ci, (q0, qn) in enumerate(qch[:2]):
                        nc.tensor.matmul(po[:65, ci * 512:ci * 512 + qn],
                                         lhsT=v_sb[kt][:krows, h, :],
                                         rhs=e_t[kt][:krows, q0:q0 + qn],
                                         start=(kt == 0), stop=(kt == NS - 1))

                scores(0)
                for kt in range(NS):
                    if kt + 1 < NS:
                        scores(kt + 1)
                    av(kt)
                    if kt % 3 == 1 and fillers:
                        fillers.pop(0)()

                # drain: out rows -> outT strip, denominator row -> den_sb
                off = (h % 2) * 64
                od = outT[h // 2][off:off + 64, :]
                dt_ = st.tile([P, S], BF16, name=f"dt{h}", tag="dt", bufs=1)
                nc.vector.tensor_copy(out=dt_[64:65, :S], in_=po[64:65, :S])
                if off == 0:
                    nc.vector.tensor_copy(out=od[:, :S], in_=po[0:64, :S])
                else:
                    # walrus requires matching partition ranges on DVE ops;
                    # odd heads hop through SBUF + DMA to reach offset 64
                    nc.vector.tensor_copy(out=dt_[0:64, :S], in_=po[0:64, :S])
                    nc.sync.dma_start(out=od[:, :S], in_=dt_[0:64, :S])
                nc.gpsimd.dma_start(out=den_d[h][0:1, :], in_=dt_[64:65, :S])
                nc.gpsimd.dma_start(out=den_sb[h:h + 1, :],
                                    in_=den_d[h][0:1, :])

            for h in range(H):
                emit_head(h)

            # batched softmax normalization: 1/den computed as exp(-ln(den))
            # on the (idle) ACT engine — the DVE reciprocal of [H, S] costs
            # ~8.6us serial; Ln+Exp cost ~2.2us and the f32 intermediate
            # lives in the now-free "po" psum.  Then broadcast via DRAM and
            # one in-place multiply per outT strip.
            po_ln = ps.tile([P, 1536], F32, name="po_ln", tag="po", bufs=1)
            nc.scalar.activation(out=po_ln[:H, :S], in_=den_sb[:H, :S],
                                 func=AFT.Ln, scale=1.0)
            with nc.allow_low_precision(reason="bf16 softmax denom"):
                nc.scalar.activation(out=den_sb[:H, :S], in_=po_ln[:H, :S],
                                     func=AFT.Exp, scale=-1.0)
            # broadcast 1/den rows to 64 partitions on the PE (rank-1 matmul
            # with a ones column) — the DMA stride-0 broadcast costs ~2us of
            # descriptor latency per strip and paced the whole proj phase
            # selector[j]: [H, P] with sel[2j+p//64, p] = 1, so
            # sel_j.T @ den_sb[0:H] replicates head (2j+half)'s recip row
            # onto partitions half*64..half*64+64 in one 16-contract matmul
            sel = cpool.tile([H, ND, P], BF16, name="sel")
            nc.vector.memset(sel, 0.0)
            ones64 = cpool.tile([1, 64], BF16, name="ones64")
            nc.vector.memset(ones64, 1.0)
            sel_d = dp.tile([1, 64], BF16, name="sel_scr", tag="selr")
            nc.sync.dma_start(out=sel_d[0:1, :], in_=ones64[0:1, :])
            for r in range(H):
                nc.sync.dma_start(
                    out=sel[r:r + 1, r // 2,
                            (r % 2) * 64:(r % 2) * 64 + 64],
                    in_=sel_d[0:1, :])
            for j in range(ND):
                for (c0, cn) in qch:
                    pb = ps.tile([P, 1024], F32, name="pbc", tag="sc",
                                 bufs=2)
                    nc.tensor.matmul(pb[:P, :cn],
                                     lhsT=sel[:, j, :],
                                     rhs=den_sb[:H, c0:c0 + cn],
                                     start=True, stop=True)
                    nc.vector.tensor_mul(out=outT[j][:, c0:c0 + cn],
                                         in0=outT[j][:, c0:c0 + cn],
                                         in1=pb[:, :cn])

            # ---------------- proj + residual + LN2 ----------------
            # LN2 emits fp8 directly (no bf16 xn2T); the fp8 transposes of
            # tile k are deferred to iteration k+2 so the PE never waits on
            # the serial proj-drain -> LN2 chain of the same tile.
            NKP1 = ND // 2                 # fc1 DoubleRow ko-pairs
            NKP2 = NM // 2                 # fc2 DoubleRow ko-pairs
            xn28T = [rp.tile([P, 2, S], F8, name=f"xn28T{j}", **RT)
                     for j in range(NKP1)]
            xn2_8s = {}

            def transp2(kt):
                s0, srows = stiles[kt]

                def wr2(j, src_ap):
                    # bf16 psum -> fp8 SBUF: the DVE copy casts for free
                    # (the PE fp8-transpose path needs stride-2 psum, so
                    # transpose in bf16 and cast on the way out)
                    nc.vector.tensor_copy(
                        out=xn28T[j // 2][:, j % 2, s0:s0 + srows],
                        in_=src_ap)
                transpose_into(xn2_8s.pop(kt), srows, s0, wr2)

            for kt, (s0, srows) in enumerate(stiles):
                x_t = st.tile([P, D], F32, name="x_t2", tag="xf", bufs=6)
                nc.gpsimd.dma_start(out=x_t[:srows], in_=x_d[s0:s0 + srows])
                x1_t = st.tile([P, D], F32, name="x1_t", tag="xf", bufs=6)
                for (n0, nn) in dch:
                    pm = ps.tile([P, 512], F32, name="ppr", tag="sc", bufs=2)
                    for ko in range(ND):
                        nc.tensor.matmul(pm[:srows, :nn],
                                         lhsT=outT[ko][:, s0:s0 + srows],
                                         rhs=projw[ko][:, n0:n0 + nn],
                                         start=(ko == 0), stop=(ko == ND - 1))
                    nc.vector.tensor_add(out=x1_t[:srows, n0:n0 + nn],
                                         in0=pm[:srows, :nn],
                                         in1=x_t[:srows, n0:n0 + nn])
                if proj_b is not None:
                    nc.vector.tensor_add(out=x1_t[:srows], in0=x1_t[:srows],
                                         in1=proj_b[:srows])
                nc.sync.dma_start(out=x1_d[s0 // P][:srows], in_=x1_t[:srows])
                xn2_8s[kt] = layer_norm(x1_t, srows, ln2_g, ln2_b,
                                        f"2_{s0}", out_pool=rp, out_kw=RT)
                if kt >= 2:
                    transp2(kt - 2)
            transp2(NS - 2)
            transp2(NS - 1)

            # fc2 weights prefetch during fc1 compute (sync queue)
            fc2w = []
            fc2_w3 = io["fc2_w"].rearrange("(ko ki) n -> ki ko n", ki=P)
            for ko in range(NM):
                t = wp.tile([P, D], BF16, name=f"fc2w{ko}", **WT)
                nc.sync.dma_start(out=t[:], in_=fc2_w3[:, ko, :])
                fc2w.append(t)

            # ---------------- fc1 -> h^T fp8 (gelu fused) ----------------
            # fp8 DoubleRow: weights x64-prescaled, xn2 unscaled e4m3; the
            # 1/64 is folded into the gelu activation scale.  gelu output is
            # written as e4m3 directly in the fc2 ko-pair layout.
            fc1_w4 = io["fc1_w"].rearrange("(kp t ki) m -> ki kp t m",
                                           ki=P, t=2)
            hT = [rp.tile([P, S], BF16, name=f"hT{mt}", **RT)
                  for mt in range(NM)]
            for mt in range(NM):
                wt = wp.tile([P, NKP1, 2, P], mybir.dt.uint8,
                             name=f"fc1w{mt}", tag="fc1w", bufs=4)
                nc.gpsimd.dma_start(out=wt[:],
                                    in_=fc1_w4[:, :, :, mt * P:(mt + 1) * P])
                for (q0, qn) in qch:
                    pm = ps.tile([P, 512], F32, name="pfc1", tag="sc", bufs=2)
                    for kp in range(NKP1):
                        nc.tensor.matmul(
                            pm[:P, :qn],
                            lhsT=wt[:, kp, :, :].bitcast(F8),
                            rhs=xn28T[kp][:, :, q0:q0 + qn],
                            start=(kp == 0), stop=(kp == NKP1 - 1),
                            perf_mode=mybir.MatmulPerfMode.DoubleRow)
                    nc.scalar.activation(
                        out=hT[mt][:, q0:q0 + qn],
                        in_=pm[:P, :qn],
                        func=AFT.Gelu if cfg.gelu else AFT.Identity,
                        bias=fc1_b_sb[:, mt:mt + 1],
                        scale=1.0 / 64.0)

            # ---------------- fc2 + residual (fp8 DoubleRow) -------------
            for (s0, srows) in stiles:
                x1_t = st.tile([P, D], F32, name="x1_t2", tag="xf", bufs=6)
                nc.gpsimd.dma_start(out=x1_t[:srows], in_=x1_d[s0 // P][:srows])
                o_t = st.tile([P, D], F32, name="o_t", tag="xf", bufs=6)
                for (n0, nn) in dch:
                    pm = ps.tile([P, 512], F32, name="pfc2", tag="sc", bufs=2)
                    for ko in range(NM):
                        nc.tensor.matmul(pm[:srows, :nn],
                                         lhsT=hT[ko][:, s0:s0 + srows],
                                         rhs=fc2w[ko][:, n0:n0 + nn],
                                         start=(ko == 0), stop=(ko == NM - 1))
                    nc.vector.tensor_add(out=o_t[:srows, n0:n0 + nn],
                                         in0=pm[:srows, :nn],
                                         in1=x1_t[:srows, n0:n0 + nn])
                if fc2_b is not None:
                    nc.vector.tensor_add(out=o_t[:srows], in0=o_t[:srows],
                                         in1=fc2_b[:srows])
                nc.sync.dma_start(out=out_d[s0:s0 + srows], in_=o_t[:srows])
    return nc


def build_full(cfg):
    nc = bacc.Bacc("TRN2", target_bir_lowering=False, debug=False)
    io = {
        "x": nc.dram_tensor("x", [cfg.S, cfg.D], F32,
                            kind="ExternalInput").ap(),
        "maskT": nc.dram_tensor("maskT", [cfg.NP, cfg.NQ], F32,
                                kind="ExternalInput").ap(),
        "qk_w": nc.dram_tensor("qk_w", [cfg.D, 2 * cfg.D],
                               mybir.dt.uint8,
                               kind="ExternalInput").ap(),
        "v_w": nc.dram_tensor("v_w", [cfg.D, cfg.D],
                              mybir.dt.uint8, kind="ExternalInput").ap(),
        "proj_w": nc.dram_tensor("proj_w", [cfg.D, cfg.D], BF16,
                                 kind="ExternalInput").ap(),
        "fc1_w": nc.dram_tensor("fc1_w", [cfg.D, cfg.MLP],
                                mybir.dt.uint8, kind="ExternalInput").ap(),
        "fc2_w": nc.dram_tensor("fc2_w", [cfg.MLP, cfg.D], BF16,
                                kind="ExternalInput").ap(),
        "fc1_b": nc.dram_tensor("fc1_b", [cfg.MLP], F32,
                                kind="ExternalInput").ap(),
        "out": nc.dram_tensor("out", [cfg.S, cfg.D], F32,
                              kind="ExternalOutput").ap(),
    }
    for flag, name in [
        (cfg.use_ln1_g, "ln1_g"), (cfg.use_ln1_b, "ln1_b"),
        (cfg.use_ln2_g, "ln2_g"), (cfg.use_ln2_b, "ln2_b"),
        (cfg.use_proj_b, "proj_b"), (cfg.use_fc2_b, "fc2_b"),
    ]:
        if flag:
            io[name] = nc.dram_tensor(name, [cfg.D], F32,
                                      kind="ExternalInput").ap()
    build_layer(nc, cfg, io)
    nc.finalize()  # runs Bacc legalization (wait splitting, regalloc)
    return nc


_CACHE = {}


def kernel(**inputs):
    x = np.asarray(inputs["x"], dtype=np.float32)
    mask = np.asarray(inputs["mask"], dtype=np.float32)
    B, S, D = x.shape
    NQ = mask.shape[1]
    NP = int(np.prod(mask.shape[2:]))
    MLP = inputs["fc1_w"].shape[1]

    cfg = Cfg(
        B=B, S=S, D=D, NP=NP, NQ=NQ, MLP=MLP,
        use_ln1_g=not np.all(np.asarray(inputs["ln1_g"]) == 1.0),
        use_ln1_b=not np.all(np.asarray(inputs["ln1_b"]) == 0.0),
        use_ln2_g=not np.all(np.asarray(inputs["ln2_g"]) == 1.0),
        use_ln2_b=not np.all(np.asarray(inputs["ln2_b"]) == 0.0),
        use_proj_b=not np.all(np.asarray(inputs["proj_b"]) == 0.0),
        use_fc2_b=not np.all(np.asarray(inputs["fc2_b"]) == 0.0),
    )
    key = cfg.key()
    if key not in _CACHE:
        _CACHE[key] = build_full(cfg)
    nc = _CACHE[key]

    bf = ml_dtypes.bfloat16
    qkv_w_f = np.asarray(inputs["qkv_w"], dtype=np.float32)
    shared = {
        "qk_w": np.ascontiguousarray(
            (qkv_w_f[:, :2 * D] * 64.0).astype(
                ml_dtypes.float8_e4m3fn)).view(np.uint8),
        "v_w": np.ascontiguousarray(
            (qkv_w_f[:, 2 * D:] * 64.0).astype(
                ml_dtypes.float8_e4m3fn)).view(np.uint8),
        "proj_w": np.ascontiguousarray(np.asarray(inputs["proj_w"]).astype(bf)),
        "fc1_w": np.ascontiguousarray(
            (np.asarray(inputs["fc1_w"], dtype=np.float32) * 64.0).astype(
                ml_dtypes.float8_e4m3fn)).view(np.uint8),
        "fc2_w": np.ascontiguousarray(np.asarray(inputs["fc2_w"]).astype(bf)),
        "fc1_b": np.ascontiguousarray(np.asarray(inputs["fc1_b"],
                                                 dtype=np.float32)),
    }
    for flag, name in [(cfg.use_ln1_g, "ln1_g"), (cfg.use_ln1_b, "ln1_b"),
                       (cfg.use_ln2_g, "ln2_g"), (cfg.use_ln2_b, "ln2_b"),
                       (cfg.use_proj_b, "proj_b"), (cfg.use_fc2_b, "fc2_b")]:
        if flag:
            shared[name] = np.ascontiguousarray(
                np.asarray(inputs[name], dtype=np.float32))

    in_maps = []
    for b in range(B):
        m = dict(shared)
        m["x"] = np.ascontiguousarray(x[b])
        m["maskT"] = np.ascontiguousarray(
            mask[b].reshape(NQ, NP).T.astype(np.float32))
        in_maps.append(m)

    from concourse.bass_utils import run_bass_kernel_spmd
    res = run_bass_kernel_spmd(nc, in_maps, core_ids=list(range(B)))
    return np.stack([res.results[b]["out"] for b in range(B)], axis=0)


if __name__ == "__main__":
    cfg = Cfg()
    nc = build_full(cfg)
    print("built ok")

